# revision 2
# baseline (speedup 1.0000x reference)
"""Trainium2 Bass kernel for nn_CrossAttention: softmax(x Wq^T (x Wk^T)^T / sqrt(C)) @ (x Wv^T).

Sharding: data-parallel over batch B=8 across the 8 NeuronCores (one batch
element per core, no collectives). Host pre-transposes/casts x -> xT and
W -> W^T (bf16); each core projects K^T and V into resident SBUF, then runs
attention in q-chunks of 256 with fp32 PSUM accumulation and fp32 softmax
statistics.
"""

import sys

sys.path.insert(0, "/opt/trn_rl_repo")

import numpy as np
import ml_dtypes

B, T, C, H = 8, 4096, 1024, 1024
NCORES = 8

CT = C // 128   # 8 contraction tiles
HT = H // 128   # 8 h tiles
ST = T // 128   # 32 key tiles
TCH = 512       # projection t-chunk
NTCH = T // TCH
QCH = 256       # attention q-chunk
NQCH = T // QCH
QS = QCH // 128  # q sub-tiles per chunk
HC = H // 512    # h chunks of 512

_CACHE = {}


def _build(reps=1):
    import concourse.bacc as bacc
    import concourse.tile as tile
    from concourse import mybir

    f32 = mybir.dt.float32
    bf16 = mybir.dt.bfloat16

    nc = bacc.Bacc("TRN2", target_bir_lowering=False, debug=False,
                   num_devices=NCORES)

    xT = nc.dram_tensor("xT", [C, T], bf16, kind="ExternalInput").ap()
    wqT = nc.dram_tensor("wqT", [C, H], bf16, kind="ExternalInput").ap()
    wkT = nc.dram_tensor("wkT", [C, H], bf16, kind="ExternalInput").ap()
    wvT = nc.dram_tensor("wvT", [C, H], bf16, kind="ExternalInput").ap()
    out = nc.dram_tensor("out", [T, H], f32, kind="ExternalOutput").ap()

    # [c, t] -> [p, a, t] with c = a*128 + p
    xTr = xT.rearrange("(a p) t -> p a t", p=128)
    wqr = wqT.rearrange("(a p) h -> p a h", p=128)
    wkr = wkT.rearrange("(a p) h -> p a h", p=128)
    wvr = wvT.rearrange("(a p) h -> p a h", p=128)

    scale = 1.0 / np.sqrt(np.float32(C))

    with tile.TileContext(nc) as tc:
        with tc.tile_pool(name="singles", bufs=1) as singles, \
             tc.tile_pool(name="xp", bufs=2) as xp, \
             tc.tile_pool(name="wp", bufs=8) as wp, \
             tc.tile_pool(name="qtp", bufs=2) as qtp, \
             tc.tile_pool(name="ptp", bufs=34) as ptp, \
             tc.tile_pool(name="accp", bufs=2) as accp, \
             tc.tile_pool(name="rcp", bufs=4) as rcp, \
             tc.tile_pool(name="op", bufs=3) as op, \
             tc.tile_pool(name="pss", bufs=4, space="PSUM") as pss, \
             tc.tile_pool(name="pso", bufs=4, space="PSUM") as pso:

            kt_sb = singles.tile([128, HT, T], bf16, tag="kt")    # K^T [h, s]
            v_sb = singles.tile([128, ST, H], bf16, tag="v")      # V [s, h]
            ones = singles.tile([128, 1], f32, tag="ones")
            nc.vector.memset(ones, 1.0)

            for rep in range(reps):
                # ---- Phase 1: project K^T and V into resident SBUF ----
                for tch in range(NTCH):
                    t0 = tch * TCH
                    xt = xp.tile([128, CT, TCH], bf16, tag="x")
                    nc.sync.dma_start(out=xt, in_=xTr[:, :, t0:t0 + TCH])
                    wk = []
                    wv = []
                    for c in range(CT):
                        wkt = wp.tile([128, H], bf16, tag="w")
                        nc.sync.dma_start(out=wkt, in_=wkr[:, c, :])
                        wk.append(wkt)
                    for c in range(CT):
                        wvt = wp.tile([128, H], bf16, tag="w")
                        nc.sync.dma_start(out=wvt, in_=wvr[:, c, :])
                        wv.append(wvt)
                    # K^T[h, t0:t0+TCH]
                    for h in range(HT):
                        ps = pso.tile([128, TCH], f32, tag="o")
                        for c in range(CT):
                            nc.tensor.matmul(ps, wk[c][:, h * 128:(h + 1) * 128],
                                             xt[:, c, :],
                                             start=(c == 0), stop=(c == CT - 1))
                        nc.scalar.copy(out=kt_sb[:, h, t0:t0 + TCH], in_=ps)
                    # V[t0:t0+TCH, :]
                    for ts in range(TCH // 128):
                        s_idx = tch * (TCH // 128) + ts
                        for k in range(HC):
                            ps = pso.tile([128, 512], f32, tag="o")
                            for c in range(CT):
                                nc.tensor.matmul(
                                    ps, xt[:, c, ts * 128:(ts + 1) * 128],
                                    wv[c][:, k * 512:(k + 1) * 512],
                                    start=(c == 0), stop=(c == CT - 1))
                            nc.vector.tensor_copy(
                                out=v_sb[:, s_idx, k * 512:(k + 1) * 512], in_=ps)

                # ---- Phase 2: attention, q-chunks of QCH ----
                for qch in range(NQCH):
                    q0 = qch * QCH
                    xq = xp.tile([128, CT, QCH], bf16, tag="x")
                    nc.sync.dma_start(out=xq, in_=xTr[:, :, q0:q0 + QCH])
                    wq = []
                    for c in range(CT):
                        wqt = wp.tile([128, H], bf16, tag="w")
                        nc.sync.dma_start(out=wqt, in_=wqr[:, c, :])
                        wq.append(wqt)
                    # Q^T chunk [h, q]
                    qt = qtp.tile([128, HT, QCH], bf16, tag="qt")
                    for h in range(HT):
                        ps = pss.tile([128, QCH], f32, tag="s")
                        for c in range(CT):
                            nc.tensor.matmul(ps, wq[c][:, h * 128:(h + 1) * 128],
                                             xq[:, c, :],
                                             start=(c == 0), stop=(c == CT - 1))
                        nc.scalar.copy(out=qt[:, h, :], in_=ps)
                    # scores S^T[s, q] = K^T.T @ Q^T ; P = exp(S * scale)
                    acc = accp.tile([128, QCH], f32, tag="acc")
                    pts = []
                    for s in range(ST):
                        ps = pss.tile([128, QCH], f32, tag="s")
                        for h in range(HT):
                            nc.tensor.matmul(ps,
                                             kt_sb[:, h, s * 128:(s + 1) * 128],
                                             qt[:, h, :],
                                             start=(h == 0), stop=(h == HT - 1))
                        pt = ptp.tile([128, QCH], bf16, tag="pt")
                        nc.scalar.activation(out=pt, in_=ps,
                                             func=mybir.ActivationFunctionType.Exp,
                                             scale=float(scale))
                        pts.append(pt)
                        if s == 0:
                            nc.vector.tensor_copy(out=acc, in_=pt)
                        else:
                            nc.vector.tensor_add(out=acc, in0=acc, in1=pt)
                    # rowsum -> reciprocal, per q sub-tile
                    recips = []
                    for j in range(QS):
                        psr = pss.tile([128, 1], f32, tag="s")
                        nc.tensor.matmul(psr, acc[:, j * 128:(j + 1) * 128], ones,
                                         start=True, stop=True)
                        rc = rcp.tile([128, 1], f32, tag="rc")
                        nc.vector.reciprocal(out=rc, in_=psr)
                        recips.append(rc)
                    # O = P^T.T @ V, accumulated over all 32 s tiles
                    pos = [[pso.tile([128, 512], f32, tag="o",
                                     name=f"po_{rep}_{qch}_{j}_{k}")
                            for k in range(HC)] for j in range(QS)]
                    for s in range(ST):
                        for j in range(QS):
                            for k in range(HC):
                                nc.tensor.matmul(
                                    pos[j][k],
                                    pts[s][:, j * 128:(j + 1) * 128],
                                    v_sb[:, s, k * 512:(k + 1) * 512],
                                    start=(s == 0), stop=(s == ST - 1))
                    for j in range(QS):
                        for k in range(HC):
                            ob = op.tile([128, 512], f32, tag="ob")
                            nc.vector.tensor_scalar_mul(ob, pos[j][k], recips[j])
                            nc.sync.dma_start(
                                out=out[q0 + j * 128:q0 + (j + 1) * 128,
                                        k * 512:(k + 1) * 512],
                                in_=ob)

    nc.compile()
    return nc


def _get_program(reps=1):
    if reps not in _CACHE:
        _CACHE[reps] = _build(reps)
    return _CACHE[reps]


def prep_inputs(x, Wq, Wk, Wv):
    """Host-side shard + layout prep: returns per-core input maps."""
    x = np.asarray(x, dtype=np.float32)
    bf = ml_dtypes.bfloat16
    wqT = np.ascontiguousarray(np.asarray(Wq, dtype=np.float32).T).astype(bf)
    wkT = np.ascontiguousarray(np.asarray(Wk, dtype=np.float32).T).astype(bf)
    wvT = np.ascontiguousarray(np.asarray(Wv, dtype=np.float32).T).astype(bf)
    in_maps = []
    for b in range(NCORES):
        xTb = np.ascontiguousarray(x[b].T).astype(bf)
        in_maps.append({"xT": xTb, "wqT": wqT, "wkT": wkT, "wvT": wvT})
    return in_maps


def kernel(x, Wq, Wk, Wv):
    from concourse import bass_utils

    in_maps = prep_inputs(x, Wq, Wk, Wv)
    nc = _get_program(reps=1)
    res = bass_utils.run_bass_kernel_spmd(nc, in_maps, list(range(NCORES)))
    return np.stack([res.results[c]["out"] for c in range(NCORES)], axis=0)


# revision 11
# speedup vs baseline: 5.1291x; 5.1291x over previous
"""Trainium2 Bass kernel for nn_CrossAttention: softmax(x Wq^T (x Wk^T)^T / sqrt(C)) @ (x Wv^T).

Sharding: data-parallel over batch B=8 across the 8 NeuronCores (one batch
element per core, no collectives). Host pre-transposes/casts x -> xT and
W -> W^T (bf16); each core projects K^T and V into resident SBUF, then runs
attention in q-chunks of 256 with fp32 PSUM accumulation and fp32 softmax
statistics. Weights stay SBUF-resident per projection phase (one shared slot).
"""

import sys

sys.path.insert(0, "/opt/trn_rl_repo")

import numpy as np
import ml_dtypes

B, T, C, H = 8, 4096, 1024, 1024
NCORES = 8

CT = C // 128   # 8 contraction tiles
HT = H // 128   # 8 h tiles
ST = T // 128   # 32 key tiles
TCH = 512       # projection t-chunk
NTCH = T // TCH
QCH = 256       # attention q-chunk
NQCH = T // QCH
QS = QCH // 128  # q sub-tiles per chunk
HC = H // 512    # h chunks of 512

_CACHE = {}


def _build(reps=1, do_p1=True, do_qt=True, do_qk=True, do_exp=True,
           do_pv=True, pss_bufs=4, pso_bufs=4, loop=False):
    import concourse.bacc as bacc
    import concourse.tile as tile
    from concourse import mybir

    f32 = mybir.dt.float32
    bf16 = mybir.dt.bfloat16

    nc = bacc.Bacc("TRN2", target_bir_lowering=False, debug=False,
                   num_devices=NCORES)

    xT = nc.dram_tensor("xT", [C, T], bf16, kind="ExternalInput").ap()
    wqT = nc.dram_tensor("wqT", [C, H], bf16, kind="ExternalInput").ap()
    wkT = nc.dram_tensor("wkT", [C, H], bf16, kind="ExternalInput").ap()
    wvT = nc.dram_tensor("wvT", [C, H], bf16, kind="ExternalInput").ap()
    out = nc.dram_tensor("out", [T, H], f32, kind="ExternalOutput").ap()

    # [c, t] -> [p, a, t] with c = a*128 + p
    xTr = xT.rearrange("(a p) t -> p a t", p=128)
    wqr = wqT.rearrange("(a p) h -> p a h", p=128)
    wkr = wkT.rearrange("(a p) h -> p a h", p=128)
    wvr = wvT.rearrange("(a p) h -> p a h", p=128)

    scale = 1.0 / np.sqrt(np.float32(C))

    with tile.TileContext(nc) as tc:
        with tc.tile_pool(name="singles", bufs=1) as singles, \
             tc.tile_pool(name="wsb", bufs=1) as wsb, \
             tc.tile_pool(name="xp", bufs=3) as xp, \
             tc.tile_pool(name="qtp", bufs=2) as qtp, \
             tc.tile_pool(name="ptp", bufs=40) as ptp, \
             tc.tile_pool(name="accp", bufs=2) as accp, \
             tc.tile_pool(name="rcp", bufs=4) as rcp, \
             tc.tile_pool(name="op", bufs=3) as op, \
             tc.tile_pool(name="pss", bufs=pss_bufs, space="PSUM") as pss, \
             tc.tile_pool(name="pso", bufs=pso_bufs, space="PSUM") as pso:

            kt_sb = singles.tile([128, HT, T], bf16, tag="kt")    # K^T [h, s]
            v_sb = singles.tile([128, ST, H], bf16, tag="v")      # V [s, h]
            ones = singles.tile([128, 1], f32, tag="ones")
            nc.vector.memset(ones, 1.0)

            def emit_rep(rep):
              if do_p1:
                # ---- Phase 1a: K^T = Wk @ xT into resident SBUF ----
                wk = wsb.tile([128, CT, H], bf16, tag="w", name=f"wk{rep}")
                nc.sync.dma_start(out=wk, in_=wkr)
                for tch in range(NTCH):
                    t0 = tch * TCH
                    xt = xp.tile([128, CT, TCH], bf16, tag="x",
                                 name=f"xtk{rep}_{tch}")
                    nc.sync.dma_start(out=xt, in_=xTr[:, :, t0:t0 + TCH])
                    for h in range(HT):
                        ps = pso.tile([128, 2, 256], f32, tag="o",
                                      name=f"psk{rep}_{tch}_{h}")
                        for half in range(2):
                            for c in range(CT):
                                nc.tensor.matmul(
                                    ps[:, half, :],
                                    wk[:, c, h * 128:(h + 1) * 128],
                                    xt[:, c, half * 256:(half + 1) * 256],
                                    start=(c == 0 and half == 0),
                                    stop=(c == CT - 1),
                                    skip_group_check=True)
                        nc.scalar.copy(
                            out=kt_sb[:, h, t0:t0 + TCH].rearrange(
                                "p (a b) -> p a b", a=2),
                            in_=ps)

                # ---- Phase 1b: V = x @ Wv^T into resident SBUF ----
                wv = wsb.tile([128, CT, H], bf16, tag="w", name=f"wv{rep}")
                nc.sync.dma_start(out=wv, in_=wvr)
                for tch in range(NTCH):
                    t0 = tch * TCH
                    xt = xp.tile([128, CT, TCH], bf16, tag="x",
                                 name=f"xtv{rep}_{tch}")
                    nc.sync.dma_start(out=xt, in_=xTr[:, :, t0:t0 + TCH])
                    for ts in range(TCH // 128):
                        s_idx = tch * (TCH // 128) + ts
                        for pair in range(HC):
                            ps = pso.tile([128, 2, 256], f32, tag="o",
                                          name=f"psv{rep}_{tch}_{ts}_{pair}")
                            for kk in range(2):
                                k = pair * 2 + kk
                                for c in range(CT):
                                    nc.tensor.matmul(
                                        ps[:, kk, :],
                                        xt[:, c, ts * 128:(ts + 1) * 128],
                                        wv[:, c, k * 256:(k + 1) * 256],
                                        start=(c == 0 and kk == 0),
                                        stop=(c == CT - 1),
                                        skip_group_check=True)
                            nc.vector.tensor_copy(
                                out=v_sb[:, s_idx,
                                         pair * 512:(pair + 1) * 512].rearrange(
                                    "p (a b) -> p a b", a=2),
                                in_=ps)

              if do_qt:
                # ---- Phase 2: attention, q-chunks of QCH ----
                wq = wsb.tile([128, CT, H], bf16, tag="w", name=f"wq{rep}")
                nc.sync.dma_start(out=wq, in_=wqr)

                def emit_qtproj(qch):
                    q0 = qch * QCH
                    xq = xp.tile([128, CT, QCH], bf16, tag="x",
                                 name=f"xq{rep}_{qch}")
                    nc.sync.dma_start(out=xq, in_=xTr[:, :, q0:q0 + QCH])
                    qt = qtp.tile([128, HT, QCH], bf16, tag="qt",
                                  name=f"qt{rep}_{qch}")
                    for h in range(HT):
                        ps = pss.tile([128, QCH], f32, tag="s",
                                      name=f"psq{rep}_{qch}_{h}")
                        for c in range(CT):
                            nc.tensor.matmul(ps,
                                             wq[:, c, h * 128:(h + 1) * 128],
                                             xq[:, c, :],
                                             start=(c == 0), stop=(c == CT - 1))
                        nc.scalar.copy(out=qt[:, h, :], in_=ps)
                    return qt

                qt_next = emit_qtproj(0) if do_qk else None
                for qch in range(NQCH):
                    q0 = qch * QCH
                    if not do_qk:
                        continue
                    qt = qt_next
                    # scores S^T[s, q] = K^T.T @ Q^T ; P = exp(S * scale)
                    acc = accp.tile([128, QCH], f32, tag="acc",
                                    name=f"acc{rep}_{qch}")
                    pts = []
                    for s in range(ST):
                        ps = pss.tile([128, QCH], f32, tag="s",
                                      name=f"pss{rep}_{qch}_{s}")
                        for h in range(HT):
                            nc.tensor.matmul(ps,
                                             kt_sb[:, h, s * 128:(s + 1) * 128],
                                             qt[:, h, :],
                                             start=(h == 0), stop=(h == HT - 1))
                        if do_exp:
                            pt = ptp.tile([128, QCH], bf16, tag="pt",
                                          name=f"pt{rep}_{qch}_{s}")
                            nc.scalar.activation(out=pt, in_=ps,
                                                 func=mybir.ActivationFunctionType.Exp,
                                                 scale=float(scale))
                            pts.append(pt)
                            if s == 0:
                                nc.vector.tensor_copy(out=acc, in_=pt)
                            else:
                                nc.vector.tensor_add(out=acc, in0=acc, in1=pt)
                    # hoisted Q^T projection for the next chunk: its PE work
                    # lands between QK and PV so ACT copies overlap PV
                    if qch + 1 < NQCH:
                        qt_next = emit_qtproj(qch + 1)
                    if not (do_pv and do_exp):
                        continue
                    # O = P^T.T @ V, accumulated over all 32 s tiles
                    pos = [[pso.tile([128, 2, 256], f32, tag="o",
                                     name=f"po{rep}_{qch}_{j}_{k}")
                            for k in range(HC)] for j in range(QS)]
                    for s in range(ST):
                        for j in range(QS):
                            for pair in range(HC):
                                for kk in range(2):
                                    nc.tensor.matmul(
                                        pos[j][pair][:, kk, :],
                                        pts[s][:, j * 128:(j + 1) * 128],
                                        v_sb[:, s,
                                             (pair * 2 + kk) * 256:
                                             (pair * 2 + kk + 1) * 256],
                                        start=(s == 0 and kk == 0),
                                        stop=(s == ST - 1),
                                        skip_group_check=True)
                    # rowsum -> reciprocal (emitted after PV: the DVE add
                    # chain finishes during PV, so PE never waits on it)
                    recips = []
                    for j in range(QS):
                        psr = pss.tile([128, 1], f32, tag="s",
                                       name=f"psr{rep}_{qch}_{j}")
                        nc.tensor.matmul(psr, acc[:, j * 128:(j + 1) * 128], ones,
                                         start=True, stop=True)
                        rc = rcp.tile([128, 1], f32, tag="rc",
                                      name=f"rc{rep}_{qch}_{j}")
                        nc.vector.reciprocal(out=rc, in_=psr)
                        recips.append(rc)
                    for j in range(QS):
                        for pair in range(HC):
                            ob = op.tile([128, 2, 256], f32, tag="ob",
                                         name=f"ob{rep}_{qch}_{j}_{pair}")
                            nc.vector.tensor_scalar_mul(ob, pos[j][pair],
                                                        recips[j])
                            nc.sync.dma_start(
                                out=out[q0 + j * 128:q0 + (j + 1) * 128,
                                        pair * 512:(pair + 1) * 512].rearrange(
                                    "p (a b) -> p a b", a=2),
                                in_=ob)

            if loop and reps > 1:
                from concourse import mybir as _mb
                engs = [_mb.EngineType.PE, _mb.EngineType.Activation,
                        _mb.EngineType.DVE, _mb.EngineType.SP]
                with tc.For_i(0, reps, 1, hint_engines=tuple(engs)):
                    emit_rep(0)
            else:
                for rep in range(reps):
                    emit_rep(rep)

    nc.compile()
    return nc


def _get_program(reps=1):
    if reps not in _CACHE:
        _CACHE[reps] = _build(reps)
    return _CACHE[reps]


def prep_inputs(x, Wq, Wk, Wv):
    """Host-side shard + layout prep: returns per-core input maps."""
    x = np.asarray(x, dtype=np.float32)
    bf = ml_dtypes.bfloat16
    wqT = np.ascontiguousarray(np.asarray(Wq, dtype=np.float32).T).astype(bf)
    wkT = np.ascontiguousarray(np.asarray(Wk, dtype=np.float32).T).astype(bf)
    wvT = np.ascontiguousarray(np.asarray(Wv, dtype=np.float32).T).astype(bf)
    in_maps = []
    for b in range(NCORES):
        xTb = np.ascontiguousarray(x[b].T).astype(bf)
        in_maps.append({"xT": xTb, "wqT": wqT, "wkT": wkT, "wvT": wvT})
    return in_maps


def kernel(x, Wq, Wk, Wv):
    from concourse import bass_utils

    in_maps = prep_inputs(x, Wq, Wk, Wv)
    nc = _get_program(reps=1)
    res = bass_utils.run_bass_kernel_spmd(nc, in_maps, list(range(NCORES)))
    return np.stack([res.results[c]["out"] for c in range(NCORES)], axis=0)
